# revision 1
# baseline (speedup 1.0000x reference)
"""Trainium2 Bass kernel for nn_ContrastiveLoss (retrieval_knn).

Computes, matching the reference:
    s = l2norm(student); t = l2norm(teacher)
    pos = sum(s*t, -1) / T
    neg_idx = neg_r + (neg_r >= idx)          # skip own index
    neg_logits[b,k] = dot(mem[neg_idx[b,k]], s[b]) / T   (mem rows are unit-norm)
    loss = mean_b( logsumexp([pos_b, neg_logits_b]) - pos_b )

Sharding: data-parallel over batch (32 rows/core on 8 cores), memory bank
replicated. Per-core kernel emits the 32 per-row (LSE - pos) values; the host
averages all 256 (== mean of per-device means with equal local batch).

The dominant cost is the gather: 4096 x 512B random rows per batch row
(64 MiB/core). Done with SWDGE indirect DMA (128 f32 per index, cast to bf16
on the fly), then DVE mul(+bf16 2x mode)/reduce, ACT exp (fused per-partition
accumulation), and a ones-matmul on PE for the cross-partition sum.
"""

import os
import sys

import numpy as np

for _p in ("/opt/trn_rl_repo",):
    if _p not in sys.path and os.path.isdir(_p):
        sys.path.insert(0, _p)

B = 256
D = 128
K = 4096
N_DATA = 1200000
TEMP = 0.07
N_CORES = 8
BLOC = B // N_CORES  # 32 batch rows per core
JCOL = K // 128      # 32 index columns per partition

_CACHE = {}
LAST_RESULTS = None


def build_nc():
    import concourse.bass as bass
    import concourse.tile as tile
    from concourse import bacc, mybir

    f32 = mybir.dt.float32
    bf16 = mybir.dt.bfloat16
    i32 = mybir.dt.int32
    Act = mybir.ActivationFunctionType
    Alu = mybir.AluOpType

    nc = bacc.Bacc("TRN2", target_bir_lowering=False, debug=False,
                   num_devices=N_CORES)

    student = nc.dram_tensor("student", [BLOC, D], f32, kind="ExternalInput").ap()
    teacher = nc.dram_tensor("teacher", [BLOC, D], f32, kind="ExternalInput").ap()
    idx = nc.dram_tensor("idx", [BLOC], i32, kind="ExternalInput").ap()
    negr = nc.dram_tensor("negr", [BLOC, K], i32, kind="ExternalInput").ap()
    mem = nc.dram_tensor("mem", [N_DATA, D], f32, kind="ExternalInput").ap()
    loss = nc.dram_tensor("loss", [1, BLOC], f32, kind="ExternalOutput").ap()
    s_scr = nc.dram_tensor("s_scratch", [BLOC, D], bf16).ap()

    with tile.TileContext(nc) as tc:
        with (
            tc.tile_pool(name="singles", bufs=1) as singles,
            tc.tile_pool(name="gpool", bufs=3) as gpool,
            tc.tile_pool(name="mpool", bufs=2) as mpool,
            tc.tile_pool(name="spool", bufs=2) as spool,
            tc.tile_pool(name="lpool", bufs=3) as lpool,
            tc.tile_pool(name="psum", bufs=1, space="PSUM") as psum,
        ):
            # ---- index prep: neg_idx = neg_r + (neg_r >= idx[b]) ----
            # negr_t[p, b*32+j] = neg_r[b, p*32+j]
            negr_t = singles.tile([128, BLOC * JCOL], i32)
            src = bass.AP(negr.tensor, 0, [[JCOL, 128], [K, BLOC], [1, JCOL]])
            dst = negr_t[:]
            dst3 = bass.AP(dst.tensor, dst.offset,
                           [dst.ap[0], [JCOL, BLOC], [1, JCOL]])
            nc.sync.dma_start(out=dst3, in_=src)

            # idxb[p, b] = idx[b] for every partition p
            idxb = singles.tile([128, BLOC], i32)
            nc.gpsimd.dma_start(
                out=idxb[:],
                in_=bass.AP(idx.tensor, 0, [[0, 128], [1, BLOC]]),
            )

            # ge = negr_t >= idxb (broadcast over j), negidx = negr_t + ge
            ge = singles.tile([128, BLOC * JCOL], i32)
            geap = ge[:]
            ge3 = bass.AP(geap.tensor, geap.offset,
                          [geap.ap[0], [JCOL, BLOC], [1, JCOL]])
            iap = idxb[:]
            idxb3 = bass.AP(iap.tensor, iap.offset,
                            [iap.ap[0], [1, BLOC], [0, JCOL]])
            nc.vector.tensor_tensor(out=ge3, in0=dst3, in1=idxb3, op=Alu.is_ge)
            negidx = singles.tile([128, BLOC * JCOL], i32)
            nc.vector.tensor_add(out=negidx[:], in0=negr_t[:], in1=ge[:])

            # ---- student / teacher normalization + pos logits ----
            s_t = singles.tile([BLOC, D], f32)
            t_t = singles.tile([BLOC, D], f32)
            nc.sync.dma_start(out=s_t[:], in_=student)
            nc.sync.dma_start(out=t_t[:], in_=teacher)

            def l2norm(x_t, name):
                sq = singles.tile([BLOC, D], f32, tag=f"sq_{name}")
                ss = singles.tile([BLOC, 1], f32, tag=f"ss_{name}")
                nc.vector.tensor_mul(out=sq[:], in0=x_t[:], in1=x_t[:])
                nc.vector.reduce_sum(out=ss[:], in_=sq[:],
                                     axis=mybir.AxisListType.X)
                nrm = singles.tile([BLOC, 1], f32, tag=f"nrm_{name}")
                nc.scalar.activation(out=nrm[:], in_=ss[:], func=Act.Sqrt)
                rn = singles.tile([BLOC, 1], f32, tag=f"rn_{name}")
                nc.vector.reciprocal(out=rn[:], in_=nrm[:])
                xn = singles.tile([BLOC, D], f32, tag=f"xn_{name}")
                nc.vector.tensor_scalar_mul(out=xn[:], in0=x_t[:], scalar1=rn[:])
                return xn

            s_n = l2norm(s_t, "s")
            t_n = l2norm(t_t, "t")

            posd = singles.tile([BLOC, D], f32)
            pos = singles.tile([BLOC, 1], f32)
            nc.vector.tensor_mul(out=posd[:], in0=s_n[:], in1=t_n[:])
            nc.vector.reduce_sum(out=pos[:], in_=posd[:],
                                 axis=mybir.AxisListType.X)

            # PP block0 col0 = pos/T, block1 col0 = exp(pos/T); stream
            # transpose flips each 32x32 block, so both land on row 0.
            PP = singles.tile([BLOC, 2 * BLOC], f32)
            nc.vector.memset(PP[:], 0.0)
            nc.scalar.activation(out=PP[:, 0:1], in_=pos[:], func=Act.Copy,
                                 scale=1.0 / TEMP)
            nc.scalar.activation(out=PP[:, BLOC:BLOC + 1], in_=pos[:],
                                 func=Act.Exp, scale=1.0 / TEMP)
            PPT = singles.tile([BLOC, 2 * BLOC], f32)
            nc.vector.transpose(out=PPT[:], in_=PP[:])

            s_bf = singles.tile([BLOC, D], bf16)
            nc.vector.tensor_copy(out=s_bf[:], in_=s_n[:])
            nc.sync.dma_start(out=s_scr, in_=s_bf[:])

            ones = singles.tile([128, 1], f32)
            nc.vector.memset(ones[:], 1.0)
            A = singles.tile([128, BLOC], f32)

            # ---- main loop over local batch rows ----
            for b in range(BLOC):
                G = gpool.tile([128, JCOL, D], bf16, tag="G")
                # One indirect DMA per 128 rows (offsets [128,1]) — the only
                # offset layout walrus unrolls correctly on HW.
                for j in range(JCOL):
                    col = b * JCOL + j
                    nc.gpsimd.indirect_dma_start(
                        out=G[:, j, :],
                        out_offset=None,
                        in_=mem,
                        in_offset=bass.IndirectOffsetOnAxis(
                            ap=negidx[:, col:col + 1], axis=0),
                    )

                s_rep = spool.tile([128, D], bf16, tag="srep")
                nc.gpsimd.dma_start(
                    out=s_rep[:],
                    in_=bass.AP(s_scr.tensor, b * D, [[0, 128], [1, D]]),
                )

                M = mpool.tile([128, JCOL, D], bf16, tag="M")
                srep_ap = s_rep[:]
                srep3 = bass.AP(srep_ap.tensor, srep_ap.offset,
                                [srep_ap.ap[0], [0, JCOL], [1, D]])
                nc.vector.tensor_mul(out=M[:], in0=G[:], in1=srep3)

                L = lpool.tile([128, JCOL, 1], f32, tag="L")
                nc.vector.reduce_sum(out=L[:], in_=M[:],
                                     axis=mybir.AxisListType.X)

                E = lpool.tile([128, JCOL], f32, tag="E")
                nc.scalar.activation(out=E[:], in_=L[:, :, 0], func=Act.Exp,
                                     scale=1.0 / TEMP,
                                     accum_out=A[:, b:b + 1])

            # ---- cross-partition sum, logsumexp, loss ----
            ps = psum.tile([1, BLOC], f32)
            nc.tensor.matmul(out=ps[:], lhsT=ones[:], rhs=A[:],
                             start=True, stop=True)
            sums_row = singles.tile([1, BLOC], f32)
            nc.vector.tensor_copy(out=sums_row[:], in_=ps[:])
            total_row = singles.tile([1, BLOC], f32)
            nc.vector.tensor_add(out=total_row[:], in0=sums_row[:],
                                 in1=PPT[0:1, BLOC:2 * BLOC])
            lse_row = singles.tile([1, BLOC], f32)
            nc.scalar.activation(out=lse_row[:], in_=total_row[:], func=Act.Ln)
            loss_row = singles.tile([1, BLOC], f32)
            nc.vector.tensor_sub(out=loss_row[:], in0=lse_row[:],
                                 in1=PPT[0:1, 0:BLOC])
            nc.sync.dma_start(out=loss, in_=loss_row[:])

    nc.compile()
    return nc


def _get_nc():
    if "nc" not in _CACHE:
        _CACHE["nc"] = build_nc()
    return _CACHE["nc"]


def kernel(student_feat, teacher_feat, indices, neg_r, memory_bank):
    global LAST_RESULTS
    from concourse.bass_utils import run_bass_kernel_spmd

    student_feat = np.asarray(student_feat, dtype=np.float32)
    teacher_feat = np.asarray(teacher_feat, dtype=np.float32)
    indices = np.asarray(indices).astype(np.int32)
    neg_r = np.asarray(neg_r).astype(np.int32)
    memory_bank = np.ascontiguousarray(np.asarray(memory_bank, dtype=np.float32))

    nc = _get_nc()
    in_maps = []
    for c in range(N_CORES):
        lo, hi = c * BLOC, (c + 1) * BLOC
        in_maps.append({
            "student": np.ascontiguousarray(student_feat[lo:hi]),
            "teacher": np.ascontiguousarray(teacher_feat[lo:hi]),
            "idx": np.ascontiguousarray(indices[lo:hi]),
            "negr": np.ascontiguousarray(neg_r[lo:hi]),
            "mem": memory_bank,
        })

    trace = bool(os.environ.get("BASS_KERNEL_TRACE"))
    res = run_bass_kernel_spmd(nc, in_maps, core_ids=list(range(N_CORES)),
                               trace=trace)
    LAST_RESULTS = res
    losses = np.concatenate([res.results[c]["loss"].reshape(-1)
                             for c in range(N_CORES)])
    return np.float32(losses.mean())

